# revision 3
# baseline (speedup 1.0000x reference)
"""IntLUTConv (1x1 conv as per-pixel GEMM) on 8 TRN2 NeuronCores.

Sharding: data-parallel over batch (B=8 -> one batch item per core), no
collectives. Per core:
  x_b [Cin=256, 16384px] fp32 -> clip(-8,7) [DVE tensor_scalar dual-op]
  -> trunc-toward-zero + cast fp8e4 [custom DVE op, magic-constant RNE +
     sign-aware correction; exact]
  -> W^T @ xq on TensorE (fp8e4, fp32 PSUM accumulation; exact integer math)
  -> ACT Copy(scale=scale/64, bias=offset) PSUM->SBUF int8 (hardware cast is
     round-half-even + saturate to [-128,127], exactly matching
     clip(round(y*scale/64 + offset), -128, 127))
  -> DMA out int8.
"""
import re
import numpy as np

import concourse.bacc as bacc
import concourse.tile as tile
import concourse.mybir as mybir
from concourse.bass_utils import run_bass_kernel_spmd
from concourse.dve_spec import Spec, Src0, Zero, C0, C1, Bin, AluOp
from concourse.dve_ops import OPS, DveOp

B, CIN, COUT, H, W = 8, 256, 256, 128, 128
NPX = H * W            # 16384 pixels per batch item
F = 1024               # pixel chunk per pipeline stage
MAGIC = 12582912.0     # 1.5 * 2**23: float add forces RNE to integer grid

TRACE = False          # test.py sets True to collect NTFF exec time
_LAST_RESULTS = [None]


def _truncq_ref(in0, in1, s0, s1, imm2):
    return np.trunc(in0)


def _register_truncq():
    for existing in OPS:
        if existing.name == "TRUNCQ_ANT":
            return existing
    t = Src0                              # pre-clipped to [-8, 7]
    a = t + C0                            # C0 = MAGIC
    r = a - C0                            # RNE(t)
    d = r - t
    q = d * t
    i = Zero < q                          # 1.0 iff rounded away from zero
    sb = Bin(AluOp.BITWISE_AND, t, C1)    # C1 = -0.0 -> sign bit of t
    c = Bin(AluOp.BITWISE_OR, i, sb)      # +/-1.0 or +/-0.0
    op = DveOp("TRUNCQ_ANT", Spec(body=r - c, reference=_truncq_ref),
               subdim=False, uops_sha={})
    OPS.append(op)
    import concourse.dve_ops as dve_ops_mod
    dve_ops_mod.CUSTOM_DVE_SPECS[op.name] = op.spec
    dve_ops_mod._SUB_OPCODE_FOR_NAME[op.name] = (
        dve_ops_mod._CUSTOM_DVE_ROW_BASE + len(OPS) - 1)
    assert dve_ops_mod._SUB_OPCODE_FOR_NAME[op.name] < 0x20
    try:
        op.compile("v3")
    except ValueError as e:
        m = re.search(r'uops_sha\["v3"\]="([0-9a-f]+)"', str(e))
        if not m:
            raise
        op.uops_sha["v3"] = m.group(1)
        op.compile("v3")
    return op


def _build(scale_val: float, offset_val: float, reps: int = 1):
    op = _register_truncq()
    nc = bacc.Bacc("TRN2", target_bir_lowering=False)
    x = nc.dram_tensor("x", [CIN, NPX], mybir.dt.float32, kind="ExternalInput")
    wt = nc.dram_tensor("wt", [CIN, COUT], mybir.dt.float8e4, kind="ExternalInput")
    out = nc.dram_tensor("out", [COUT, NPX], mybir.dt.int8, kind="ExternalOutput")

    with tile.TileContext(nc) as tc, \
         tc.tile_pool(name="singles", bufs=1) as singles, \
         tc.tile_pool(name="work", bufs=3) as work, \
         tc.tile_pool(name="outs", bufs=3) as outs, \
         tc.tile_pool(name="psum", bufs=4, space="PSUM") as pspool:
        wt_sb = []
        for ct in range(2):
            w_t = singles.tile([128, COUT], mybir.dt.float8e4, tag=f"wt{ct}")
            nc.sync.dma_start(out=w_t[:, :], in_=wt[ct * 128:(ct + 1) * 128, :])
            wt_sb.append(w_t)

        for i in [i for _ in range(reps) for i in range(NPX // F)]:
            xqs = []
            for ct in range(2):
                rows = slice(ct * 128, (ct + 1) * 128)
                cols = slice(i * F, (i + 1) * F)
                xr = work.tile([128, F], mybir.dt.float32, tag=f"xr{ct}")
                nc.sync.dma_start(out=xr[:, :], in_=x[rows, cols])
                tcl = work.tile([128, F], mybir.dt.float32, tag=f"tcl{ct}")
                nc.vector.tensor_scalar(
                    out=tcl[:, :], in0=xr[:, :], scalar1=7.0, scalar2=-8.0,
                    op0=mybir.AluOpType.min, op1=mybir.AluOpType.max,
                )
                xq = work.tile([128, F], mybir.dt.float8e4, tag=f"xq{ct}")
                nc.vector._custom_dve(op, out=xq[:, :], in0=tcl[:, :],
                                      s0=MAGIC, s1=-0.0)
                xqs.append(xq)
            for o in range(2):
                ps = pspool.tile([128, F], mybir.dt.float32, tag="ps")
                for ct in range(2):
                    for sub in range(F // 512):
                        nc.tensor.matmul(
                            ps[:, sub * 512:(sub + 1) * 512],
                            wt_sb[ct][:, o * 128:(o + 1) * 128],
                            xqs[ct][:, sub * 512:(sub + 1) * 512],
                            start=(ct == 0), stop=(ct == 1),
                        )
                oi8 = outs.tile([128, F], mybir.dt.int8, tag=f"oi8{o}")
                nc.scalar.activation(
                    out=oi8[:, :], in_=ps[:, :],
                    func=mybir.ActivationFunctionType.Copy,
                    scale=scale_val / 64.0, bias=offset_val,
                )
                nc.sync.dma_start(out=out[o * 128:(o + 1) * 128, i * F:(i + 1) * F],
                                  in_=oi8[:, :])
    nc.finalize()
    return nc


_KERNEL_CACHE: dict = {}


def kernel(x, weights, scale, offset):
    x = np.asarray(x)
    weights = np.asarray(weights)
    sv = float(np.asarray(scale))
    ov = float(np.asarray(offset))

    key = (sv, ov)
    if key not in _KERNEL_CACHE:
        _KERNEL_CACHE[key] = _build(sv, ov)
    nc = _KERNEL_CACHE[key]

    dt_f8 = mybir.dt.np(mybir.dt.float8e4)
    wt_host = np.ascontiguousarray(weights.T).astype(np.float32).astype(dt_f8)

    in_maps = [
        {"x": np.ascontiguousarray(x[b].reshape(CIN, NPX)), "wt": wt_host}
        for b in range(B)
    ]
    res = run_bass_kernel_spmd(nc, in_maps, core_ids=list(range(B)),
                               trace=TRACE)
    _LAST_RESULTS[0] = res
    return np.stack([r["out"].reshape(COUT, H, W) for r in res.results])
